# revision 36
# baseline (speedup 1.0000x reference)
"""BoundaryMaxPooling Trainium2 kernel (v3: bf16 PE sparse-table).

Reference computation (B=16, C2=512, T=Tf=126):
  - segment windows [s0,s1) / [e0,e1) derived from segments[0] only
  - out[b, c, t]      = max_{j in [s0(t), s1(t))} feature[b, c, j]       (c < 256)
  - out[b, 256+c, t]  = max_{j in [e0(t), e1(t))} feature[b, 256+c, j]

Device algorithm (per core, 2 batches, data-parallel over batch), all in
bf16 (rel tol 2e-2 >> bf16's 4e-3):
  Sparse-table (log-level) range max with time j on SBUF partitions:
    L_0[j, c'] = feature^T   (c' = half*512 + b*256 + c, 1024 columns)
    L_{k+1}[j] = max(L_k[j], L_k[j + 2^k])
  Partition shifts and the two per-window lookups (columns lo and
  hi - 2^k of level floor(log2 len)) are exact one-hot bf16 matmuls on
  the PE (1 cycle/row), accumulated over levels in f32 PSUM; host
  precomputes the one-hot matrices from segments[0] (replicated across
  cores; level-0 windows have len 1 so both lookups share one matrix).
  The feature + level-0 weights share one SP-queue DMA (level 0 gates
  the first real pass); later levels stream on the ACT/Pool queues in
  consumption order.  A run of zero matmuls keeps the PE continuously
  busy from kernel start so HAM has it at full clock when the data
  lands.  The final max(acc_a, acc_b) stages acc_a through SBUF on the
  ACT engine (DVE reads one PSUM operand per op) while the DVE merges;
  host
  casts to f32, reassembles, and fills empty end-windows with f32 min
  (data-independent), matching the reference.
"""

import os
import sys

import numpy as np
import ml_dtypes

if os.path.isdir("/opt/trn_rl_repo") and "/opt/trn_rl_repo" not in sys.path:
    sys.path.insert(0, "/opt/trn_rl_repo")

import concourse.bass as bass  # noqa: E402
from concourse import bacc, mybir, tile  # noqa: E402
from concourse.bass_utils import run_bass_kernel_spmd  # noqa: E402

B, C2, T = 16, 512, 126
C = C2 // 2  # 256
NCORES = 8
BPC = B // NCORES  # batches per core = 2
CPRIME = BPC * C2  # 1024 columns per core
NLEV = 7
KS = [127 - (1 << k) for k in range(NLEV)]  # valid rows of level k

BF16 = mybir.dt.bfloat16
F32 = mybir.dt.float32

N_WARMUP = 7  # zero matmuls bridging kernel start -> first weights

_CACHE = {}

TRACE = False
LAST_RESULTS = None

# chunk c holds, per level k in CHUNK_LEVELS[c]: [sh_k | g(0,0,k) g(1,0,k)
# g(0,1,k) g(1,1,k)]  (sh only for k < 6)
CHUNK_LEVELS = [[0], [1, 2], [3, 4, 5, 6]]


def _wts_layout():
    """ft chunk + three weight chunks (one DMA each, ordered by level)."""
    offs = {}
    totals = []
    for levels in CHUNK_LEVELS:
        off = 0
        ci = len(totals) + 1
        for k in levels:
            if k < NLEV - 1:
                offs[("sh", k)] = (ci, off, KS[k + 1])
                off += KS[k + 1]
            for gi in range(2):
                for h in range(2):
                    if k == 0 and gi == 1:
                        # level-0 windows have len 1, so both lookups hit
                        # the same column: gi=1 reuses gi=0's one-hot
                        continue
                    offs[("g", gi, h, k)] = (ci, off, T)
                    off += T
        totals.append(off)
    return offs, totals


def _build_module():
    nc = bacc.Bacc(None, target_bir_lowering=False, debug=False)

    offs, totals = _wts_layout()
    # level-0 weights ride in the same DMA as the feature (bigger rows
    # stream faster, and level 0 gates the first real matmul)
    ft = nc.dram_tensor("ft", [T, CPRIME + totals[0]], BF16, kind="ExternalInput")
    wts = [
        nc.dram_tensor(f"w{c}", [T, totals[c]], BF16, kind="ExternalInput")
        for c in range(1, 3)
    ]
    out = nc.dram_tensor("out", [T, CPRIME], BF16, kind="ExternalOutput")

    with tile.TileContext(nc) as tc:
        with (
            tc.tile_pool(name="lv", bufs=1) as lvp,
            tc.tile_pool(name="gw", bufs=1) as gwp,
            tc.tile_pool(name="acc", bufs=1, space=bass.MemorySpace.PSUM) as accp,
            tc.tile_pool(name="shp", bufs=4, space=bass.MemorySpace.PSUM) as shpp,
        ):
            ftw = gwp.tile([T, CPRIME + totals[0]], BF16, name="ftw")
            wt12 = [
                gwp.tile([T, totals[c]], BF16, name=f"wt{c}") for c in (1, 2)
            ]
            wt = [ftw, wt12[0], wt12[1]]  # chunk index -> tile
            woff = [CPRIME, 0, 0]  # chunk base column within its tile
            # ft + level-0 weights in one SP-queue DMA (bigger rows stream
            # faster and level 0 gates the first real matmul); w1 on the
            # ACT queue, w2 (consumed last) on the Pool queue.
            nc.sync.dma_start(out=ftw[:, :], in_=ft[:, :])
            nc.scalar.dma_start(out=wt12[0][:, :], in_=wts[0][:, :])
            nc.gpsimd.dma_start(out=wt12[1][:, :], in_=wts[1][:, :])

            L = [ftw[:, 0:CPRIME]] + [
                lvp.tile([KS[k], CPRIME], BF16, name=f"L{k}")[:, :]
                for k in range(1, NLEV)
            ]

            def sh_ap(k):
                ci, o, n = offs[("sh", k)]
                o += woff[ci - 1]
                return wt[ci - 1][0 : KS[k], o : o + n]

            def g_ap(gi, h, k):
                if k == 0:
                    gi = 0  # level-0 one-hots are identical for both lookups
                ci, o, n = offs[("g", gi, h, k)]
                o += woff[ci - 1]
                return wt[ci - 1][0 : KS[k], o : o + n]

            p_acc = [
                accp.tile([T, CPRIME], F32, name=f"pacc{gi}") for gi in range(2)
            ]

            # PE warmup: HAM runs the PE at reduced clock until ~3us of
            # continuous busy; zero bf16 matmuls bridge the gap between
            # kernel start and the first weight DMA landing so the real
            # matmuls run at full clock with no idle gap.
            wzero = gwp.tile([128, 512], BF16, name="wzero")
            nc.vector.memset(wzero[:, :], 0.0)
            for _ in range(N_WARMUP):
                nc.tensor.matmul(
                    p_acc[0][:, 0:512],
                    wzero[0:126, 0:126],
                    wzero[0:126, 0:512],
                    start=True,
                    stop=True,
                )

            # per level: shift both halves first (PE), gathers accumulate,
            # DVE folds the shifted rows into the next level meanwhile.
            for k in range(NLEV):
                pe_shift = k < NLEV - 1
                for h in range(2) if k < NLEV - 1 else ():
                    sl = slice(h * 512, (h + 1) * 512)
                    if pe_shift:
                        shp = shpp.tile(
                            [KS[k + 1], 512], F32, name=f"shp{k}{h}", tag="shp"
                        )
                        nc.tensor.matmul(
                            shp[:, :],
                            sh_ap(k),
                            L[k][:, sl],
                            start=True,
                            stop=True,
                        )
                    if k < NLEV - 1:
                        for gi in range(2):
                            nc.tensor.matmul(
                                p_acc[gi][:, sl],
                                g_ap(gi, h, k),
                                L[k][:, sl],
                                start=(k == 0),
                                stop=False,
                            )
                    if pe_shift:
                        nc.vector.tensor_max(
                            L[k + 1][:, sl],
                            L[k][0 : KS[k + 1], sl],
                            shp[:, :],
                        )
                if k == NLEV - 1:
                    # last level, gi-major: acc0's final writes land first so
                    # the DVE staging op can start under the acc1 passes
                    for gi in range(2):
                        for h in range(2):
                            sl = slice(h * 512, (h + 1) * 512)
                            nc.tensor.matmul(
                                p_acc[gi][:, sl],
                                g_ap(gi, h, k),
                                L[k][:, sl],
                                start=False,
                                stop=True,
                            )

            # final per half (DVE can read only one PSUM operand per op):
            # stage acc0 to SBUF bf16 via tensor_scalar_max, then max with
            # acc1; one output DMA per half.
            s1t = gwp.tile([T, CPRIME], BF16, name="s1t")
            ot = gwp.tile([T, CPRIME], BF16, name="ot")
            for half in range(2):
                sl = slice(half * 512, (half + 1) * 512)
                nc.scalar.copy(s1t[:, sl], p_acc[0][:, sl])
            for half in range(2):
                sl = slice(half * 512, (half + 1) * 512)
                nc.vector.tensor_max(ot[:, sl], s1t[:, sl], p_acc[1][:, sl])
                e0 = nc.sync if half == 0 else nc.scalar
                e1 = nc.gpsimd if half == 0 else nc.sync
                e0.dma_start(out=out[0:63, sl], in_=ot[0:63, sl])
                e1.dma_start(out=out[63:126, sl], in_=ot[63:126, sl])

    nc.compile()
    return nc


def _host_windows(segments):
    """Replicates the reference's index math on segments[0]. Returns per half
    (lo, hi) clamped windows plus the empty mask."""
    seg = np.clip(segments.astype(np.float32), 0.0, 125.0)
    row = seg[0]  # [T, 4]
    s0 = np.floor(row[:, 0]).astype(np.int32)
    s1 = np.ceil(row[:, 1]).astype(np.int32)
    s1 = np.where(s0 == s1, s1 + 1, s1)
    e0 = np.floor(row[:, 2]).astype(np.int32)
    e1 = np.ceil(row[:, 3]).astype(np.int32)
    e0 = np.where(e0 == e1, e0 - 1, e0)

    halves = []
    for lo, hi in ((s0, s1), (e0, e1)):
        lo_c = np.maximum(lo, 0)
        hi_c = np.minimum(hi, T)
        empty = lo_c >= hi_c
        halves.append((lo_c, hi_c, empty))
    return halves


def _host_matrices(segments):
    halves = _host_windows(segments)
    g = {
        (gi, h, k): np.zeros((KS[k], T), np.float32)
        for gi in range(2)
        for h in range(2)
        for k in range(NLEV)
    }
    for h, (lo, hi, empty) in enumerate(halves):
        for t in range(T):
            if empty[t]:
                continue
            ln = int(hi[t] - lo[t])
            k = ln.bit_length() - 1
            g[(0, h, k)][int(lo[t]), t] = 1.0
            g[(1, h, k)][int(hi[t]) - (1 << k), t] = 1.0
    sh = {}
    for k in range(NLEV - 1):
        m = np.zeros((KS[k], KS[k + 1]), np.float32)
        s = 1 << k
        for j in range(KS[k + 1]):
            m[j + s, j] = 1.0
        sh[k] = m
    return g, sh, halves


def _shard_feature(feature):
    """Core i gets batches [2i, 2i+2) as [T, CPRIME] bf16 with
    c' = half*512 + local_batch*256 + channel_within_half."""
    fts = []
    for i in range(NCORES):
        pair = feature[BPC * i : BPC * (i + 1)]
        arr = pair.reshape(BPC, 2, C, T)  # [b, h, c, j]
        arr = np.ascontiguousarray(arr.transpose(3, 1, 0, 2).reshape(T, CPRIME))
        fts.append(arr.astype(ml_dtypes.bfloat16))
    return fts


def _unshard(results, halves):
    out = np.empty((B, C2, T), np.float32)
    for i in range(NCORES):
        r = np.asarray(results[i]["out"]).astype(np.float32)  # [T, CPRIME]
        arr = r.reshape(T, 2, BPC, C).transpose(2, 1, 3, 0)  # [b, h, c, t]
        out[BPC * i : BPC * (i + 1)] = arr.reshape(BPC, C2, T)
    neg = np.finfo(np.float32).min
    for h, (_, _, empty) in enumerate(halves):
        if empty.any():
            out[:, h * C : (h + 1) * C, empty] = neg
    return out


def kernel(feature, segments):
    global LAST_RESULTS
    feature = np.ascontiguousarray(feature, dtype=np.float32)
    segments = np.ascontiguousarray(segments, dtype=np.float32)

    g, sh, halves = _host_matrices(segments)
    if "nc" not in _CACHE:
        _CACHE["nc"] = _build_module()
    nc = _CACHE["nc"]

    fts = _shard_feature(feature)

    offs, totals = _wts_layout()
    chunks = [np.zeros((T, totals[c]), np.float32) for c in range(3)]
    for k in range(NLEV):
        if k < NLEV - 1:
            ci, o, n = offs[("sh", k)]
            chunks[ci - 1][: KS[k], o : o + n] = sh[k]
        for gi in range(2):
            for h in range(2):
                if k == 0 and gi == 1:
                    continue
                ci, o, n = offs[("g", gi, h, k)]
                chunks[ci - 1][: KS[k], o : o + n] = g[(gi, h, k)]
    chunks = [c.astype(ml_dtypes.bfloat16) for c in chunks]
    in_maps = []
    for i in range(NCORES):
        m = {
            "ft": np.ascontiguousarray(np.concatenate([fts[i], chunks[0]], axis=1)),
            "w1": chunks[1],
            "w2": chunks[2],
        }
        in_maps.append(m)

    res = run_bass_kernel_spmd(nc, in_maps, list(range(NCORES)), trace=TRACE)
    LAST_RESULTS = res
    return _unshard(res.results, halves)


# revision 37
# speedup vs baseline: 1.0446x; 1.0446x over previous
"""BoundaryMaxPooling Trainium2 kernel (v3: bf16 PE sparse-table).

Reference computation (B=16, C2=512, T=Tf=126):
  - segment windows [s0,s1) / [e0,e1) derived from segments[0] only
  - out[b, c, t]      = max_{j in [s0(t), s1(t))} feature[b, c, j]       (c < 256)
  - out[b, 256+c, t]  = max_{j in [e0(t), e1(t))} feature[b, 256+c, j]

Device algorithm (per core, 2 batches, data-parallel over batch), all in
bf16 (rel tol 2e-2 >> bf16's 4e-3):
  Sparse-table (log-level) range max with time j on SBUF partitions:
    L_0[j, c'] = feature^T   (c' = half*512 + b*256 + c, 1024 columns)
    L_{k+1}[j] = max(L_k[j], L_k[j + 2^k])
  Partition shifts and the two per-window lookups (columns lo and
  hi - 2^k of level floor(log2 len)) are exact one-hot bf16 matmuls on
  the PE (1 cycle/row), accumulated over levels in f32 PSUM; host
  precomputes the one-hot matrices from segments[0] (replicated across
  cores; level-0 windows have len 1 so both lookups share one matrix).
  The feature + level-0 weights share one SP-queue DMA (level 0 gates
  the first real pass); later levels stream on the ACT/Pool queues in
  consumption order.  A run of zero matmuls keeps the PE continuously
  busy from kernel start so HAM has it at full clock when the data
  lands.  The final max(acc_a, acc_b) stages acc_a through SBUF on the
  ACT engine (DVE reads one PSUM operand per op) while the DVE merges;
  host
  casts to f32, reassembles, and fills empty end-windows with f32 min
  (data-independent), matching the reference.
"""

import os
import sys

import numpy as np
import ml_dtypes

if os.path.isdir("/opt/trn_rl_repo") and "/opt/trn_rl_repo" not in sys.path:
    sys.path.insert(0, "/opt/trn_rl_repo")

import concourse.bass as bass  # noqa: E402
from concourse import bacc, mybir, tile  # noqa: E402
from concourse.bass_utils import run_bass_kernel_spmd  # noqa: E402

B, C2, T = 16, 512, 126
C = C2 // 2  # 256
NCORES = 8
BPC = B // NCORES  # batches per core = 2
CPRIME = BPC * C2  # 1024 columns per core
NLEV = 7
KS = [127 - (1 << k) for k in range(NLEV)]  # valid rows of level k

BF16 = mybir.dt.bfloat16
F32 = mybir.dt.float32

N_WARMUP = 9  # zero matmuls bridging kernel start -> first weights

_CACHE = {}

TRACE = False
LAST_RESULTS = None

# chunk c holds, per level k in CHUNK_LEVELS[c]: [sh_k | g(0,0,k) g(1,0,k)
# g(0,1,k) g(1,1,k)]  (sh only for k < 6)
CHUNK_LEVELS = [[0], [1, 2], [3, 4, 5, 6]]


def _wts_layout():
    """ft chunk + three weight chunks (one DMA each, ordered by level)."""
    offs = {}
    totals = []
    for levels in CHUNK_LEVELS:
        off = 0
        ci = len(totals) + 1
        for k in levels:
            if k < NLEV - 1:
                offs[("sh", k)] = (ci, off, KS[k + 1])
                off += KS[k + 1]
            for gi in range(2):
                for h in range(2):
                    if k == 0 and gi == 1:
                        # level-0 windows have len 1, so both lookups hit
                        # the same column: gi=1 reuses gi=0's one-hot
                        continue
                    offs[("g", gi, h, k)] = (ci, off, T)
                    off += T
        totals.append(off)
    return offs, totals


def _build_module():
    nc = bacc.Bacc(None, target_bir_lowering=False, debug=False)

    offs, totals = _wts_layout()
    # level-0 weights ride in the same DMA as the feature (bigger rows
    # stream faster, and level 0 gates the first real matmul)
    ft = nc.dram_tensor("ft", [T, CPRIME + totals[0]], BF16, kind="ExternalInput")
    wts = [
        nc.dram_tensor(f"w{c}", [T, totals[c]], BF16, kind="ExternalInput")
        for c in range(1, 3)
    ]
    out = nc.dram_tensor("out", [T, CPRIME], BF16, kind="ExternalOutput")

    with tile.TileContext(nc) as tc:
        with (
            tc.tile_pool(name="lv", bufs=1) as lvp,
            tc.tile_pool(name="gw", bufs=1) as gwp,
            tc.tile_pool(name="acc", bufs=1, space=bass.MemorySpace.PSUM) as accp,
            tc.tile_pool(name="shp", bufs=4, space=bass.MemorySpace.PSUM) as shpp,
        ):
            ftw = gwp.tile([T, CPRIME + totals[0]], BF16, name="ftw")
            wt12 = [
                gwp.tile([T, totals[c]], BF16, name=f"wt{c}") for c in (1, 2)
            ]
            wt = [ftw, wt12[0], wt12[1]]  # chunk index -> tile
            woff = [CPRIME, 0, 0]  # chunk base column within its tile
            # ft + level-0 weights in one SP-queue DMA (bigger rows stream
            # faster and level 0 gates the first real matmul); w1 on the
            # ACT queue, w2 (consumed last) on the Pool queue.
            nc.sync.dma_start(out=ftw[:, :], in_=ft[:, :])
            nc.scalar.dma_start(out=wt12[0][:, :], in_=wts[0][:, :])
            nc.gpsimd.dma_start(out=wt12[1][:, :], in_=wts[1][:, :])

            L = [ftw[:, 0:CPRIME]] + [
                lvp.tile([KS[k], CPRIME], BF16, name=f"L{k}")[:, :]
                for k in range(1, NLEV)
            ]

            def sh_ap(k):
                ci, o, n = offs[("sh", k)]
                o += woff[ci - 1]
                return wt[ci - 1][0 : KS[k], o : o + n]

            def g_ap(gi, h, k):
                if k == 0:
                    gi = 0  # level-0 one-hots are identical for both lookups
                ci, o, n = offs[("g", gi, h, k)]
                o += woff[ci - 1]
                return wt[ci - 1][0 : KS[k], o : o + n]

            p_acc = [
                accp.tile([T, CPRIME], F32, name=f"pacc{gi}") for gi in range(2)
            ]

            # PE warmup: HAM runs the PE at reduced clock until ~3us of
            # continuous busy; zero bf16 matmuls bridge the gap between
            # kernel start and the first weight DMA landing so the real
            # matmuls run at full clock with no idle gap.
            wzero = gwp.tile([128, 512], BF16, name="wzero")
            nc.vector.memset(wzero[:, :], 0.0)
            for _ in range(N_WARMUP):
                nc.tensor.matmul(
                    p_acc[0][:, 0:512],
                    wzero[0:126, 0:126],
                    wzero[0:126, 0:512],
                    start=True,
                    stop=True,
                )

            # per level: shift both halves first (PE), gathers accumulate,
            # DVE folds the shifted rows into the next level meanwhile.
            for k in range(NLEV):
                pe_shift = k < NLEV - 1
                for h in range(2) if k < NLEV - 1 else ():
                    sl = slice(h * 512, (h + 1) * 512)
                    if pe_shift:
                        shp = shpp.tile(
                            [KS[k + 1], 512], F32, name=f"shp{k}{h}", tag="shp"
                        )
                        nc.tensor.matmul(
                            shp[:, :],
                            sh_ap(k),
                            L[k][:, sl],
                            start=True,
                            stop=True,
                        )
                    if k < NLEV - 1:
                        for gi in range(2):
                            nc.tensor.matmul(
                                p_acc[gi][:, sl],
                                g_ap(gi, h, k),
                                L[k][:, sl],
                                start=(k == 0),
                                stop=False,
                            )
                    if pe_shift:
                        nc.vector.tensor_max(
                            L[k + 1][:, sl],
                            L[k][0 : KS[k + 1], sl],
                            shp[:, :],
                        )
                if k == NLEV - 1:
                    # last level, gi-major: acc0's final writes land first so
                    # the DVE staging op can start under the acc1 passes
                    for gi in range(2):
                        for h in range(2):
                            sl = slice(h * 512, (h + 1) * 512)
                            nc.tensor.matmul(
                                p_acc[gi][:, sl],
                                g_ap(gi, h, k),
                                L[k][:, sl],
                                start=False,
                                stop=True,
                            )

            # final per half (DVE can read only one PSUM operand per op):
            # stage acc0 to SBUF bf16 via tensor_scalar_max, then max with
            # acc1; one output DMA per half.
            s1t = gwp.tile([T, CPRIME], BF16, name="s1t")
            ot = gwp.tile([T, CPRIME], BF16, name="ot")
            for half in range(2):
                sl = slice(half * 512, (half + 1) * 512)
                nc.scalar.copy(s1t[:, sl], p_acc[0][:, sl])
            for half in range(2):
                sl = slice(half * 512, (half + 1) * 512)
                nc.vector.tensor_max(ot[:, sl], s1t[:, sl], p_acc[1][:, sl])
                e0 = nc.sync if half == 0 else nc.scalar
                e1 = nc.gpsimd if half == 0 else nc.sync
                e0.dma_start(out=out[0:63, sl], in_=ot[0:63, sl])
                e1.dma_start(out=out[63:126, sl], in_=ot[63:126, sl])

    nc.compile()
    return nc


def _host_windows(segments):
    """Replicates the reference's index math on segments[0]. Returns per half
    (lo, hi) clamped windows plus the empty mask."""
    seg = np.clip(segments.astype(np.float32), 0.0, 125.0)
    row = seg[0]  # [T, 4]
    s0 = np.floor(row[:, 0]).astype(np.int32)
    s1 = np.ceil(row[:, 1]).astype(np.int32)
    s1 = np.where(s0 == s1, s1 + 1, s1)
    e0 = np.floor(row[:, 2]).astype(np.int32)
    e1 = np.ceil(row[:, 3]).astype(np.int32)
    e0 = np.where(e0 == e1, e0 - 1, e0)

    halves = []
    for lo, hi in ((s0, s1), (e0, e1)):
        lo_c = np.maximum(lo, 0)
        hi_c = np.minimum(hi, T)
        empty = lo_c >= hi_c
        halves.append((lo_c, hi_c, empty))
    return halves


def _host_matrices(segments):
    halves = _host_windows(segments)
    g = {
        (gi, h, k): np.zeros((KS[k], T), np.float32)
        for gi in range(2)
        for h in range(2)
        for k in range(NLEV)
    }
    for h, (lo, hi, empty) in enumerate(halves):
        for t in range(T):
            if empty[t]:
                continue
            ln = int(hi[t] - lo[t])
            k = ln.bit_length() - 1
            g[(0, h, k)][int(lo[t]), t] = 1.0
            g[(1, h, k)][int(hi[t]) - (1 << k), t] = 1.0
    sh = {}
    for k in range(NLEV - 1):
        m = np.zeros((KS[k], KS[k + 1]), np.float32)
        s = 1 << k
        for j in range(KS[k + 1]):
            m[j + s, j] = 1.0
        sh[k] = m
    return g, sh, halves


def _shard_feature(feature):
    """Core i gets batches [2i, 2i+2) as [T, CPRIME] bf16 with
    c' = half*512 + local_batch*256 + channel_within_half."""
    fts = []
    for i in range(NCORES):
        pair = feature[BPC * i : BPC * (i + 1)]
        arr = pair.reshape(BPC, 2, C, T)  # [b, h, c, j]
        arr = np.ascontiguousarray(arr.transpose(3, 1, 0, 2).reshape(T, CPRIME))
        fts.append(arr.astype(ml_dtypes.bfloat16))
    return fts


def _unshard(results, halves):
    out = np.empty((B, C2, T), np.float32)
    for i in range(NCORES):
        r = np.asarray(results[i]["out"]).astype(np.float32)  # [T, CPRIME]
        arr = r.reshape(T, 2, BPC, C).transpose(2, 1, 3, 0)  # [b, h, c, t]
        out[BPC * i : BPC * (i + 1)] = arr.reshape(BPC, C2, T)
    neg = np.finfo(np.float32).min
    for h, (_, _, empty) in enumerate(halves):
        if empty.any():
            out[:, h * C : (h + 1) * C, empty] = neg
    return out


def kernel(feature, segments):
    global LAST_RESULTS
    feature = np.ascontiguousarray(feature, dtype=np.float32)
    segments = np.ascontiguousarray(segments, dtype=np.float32)

    g, sh, halves = _host_matrices(segments)
    if "nc" not in _CACHE:
        _CACHE["nc"] = _build_module()
    nc = _CACHE["nc"]

    fts = _shard_feature(feature)

    offs, totals = _wts_layout()
    chunks = [np.zeros((T, totals[c]), np.float32) for c in range(3)]
    for k in range(NLEV):
        if k < NLEV - 1:
            ci, o, n = offs[("sh", k)]
            chunks[ci - 1][: KS[k], o : o + n] = sh[k]
        for gi in range(2):
            for h in range(2):
                if k == 0 and gi == 1:
                    continue
                ci, o, n = offs[("g", gi, h, k)]
                chunks[ci - 1][: KS[k], o : o + n] = g[(gi, h, k)]
    chunks = [c.astype(ml_dtypes.bfloat16) for c in chunks]
    in_maps = []
    for i in range(NCORES):
        m = {
            "ft": np.ascontiguousarray(np.concatenate([fts[i], chunks[0]], axis=1)),
            "w1": chunks[1],
            "w2": chunks[2],
        }
        in_maps.append(m)

    res = run_bass_kernel_spmd(nc, in_maps, list(range(NCORES)), trace=TRACE)
    LAST_RESULTS = res
    return _unshard(res.results, halves)


# revision 38
# speedup vs baseline: 1.0595x; 1.0143x over previous
"""BoundaryMaxPooling Trainium2 kernel (v3: bf16 PE sparse-table).

Reference computation (B=16, C2=512, T=Tf=126):
  - segment windows [s0,s1) / [e0,e1) derived from segments[0] only
  - out[b, c, t]      = max_{j in [s0(t), s1(t))} feature[b, c, j]       (c < 256)
  - out[b, 256+c, t]  = max_{j in [e0(t), e1(t))} feature[b, 256+c, j]

Device algorithm (per core, 2 batches, data-parallel over batch), all in
bf16 (rel tol 2e-2 >> bf16's 4e-3):
  Sparse-table (log-level) range max with time j on SBUF partitions:
    L_0[j, c'] = feature^T   (c' = half*512 + b*256 + c, 1024 columns)
    L_{k+1}[j] = max(L_k[j], L_k[j + 2^k])
  Partition shifts and the two per-window lookups (columns lo and
  hi - 2^k of level floor(log2 len)) are exact one-hot bf16 matmuls on
  the PE (1 cycle/row), accumulated over levels in f32 PSUM; host
  precomputes the one-hot matrices from segments[0] (replicated across
  cores; level-0 windows have len 1 so both lookups share one matrix).
  The feature + level-0 weights share one SP-queue DMA (level 0 gates
  the first real pass); later levels stream on the ACT/Pool queues in
  consumption order.  A run of zero matmuls keeps the PE continuously
  busy from kernel start so HAM has it at full clock when the data
  lands.  The final max(acc_a, acc_b) stages acc_a through SBUF on the
  ACT engine (DVE reads one PSUM operand per op) while the DVE merges;
  host
  casts to f32, reassembles, and fills empty end-windows with f32 min
  (data-independent), matching the reference.
"""

import os
import sys

import numpy as np
import ml_dtypes

if os.path.isdir("/opt/trn_rl_repo") and "/opt/trn_rl_repo" not in sys.path:
    sys.path.insert(0, "/opt/trn_rl_repo")

import concourse.bass as bass  # noqa: E402
from concourse import bacc, mybir, tile  # noqa: E402
from concourse.bass_utils import run_bass_kernel_spmd  # noqa: E402

B, C2, T = 16, 512, 126
C = C2 // 2  # 256
NCORES = 8
BPC = B // NCORES  # batches per core = 2
CPRIME = BPC * C2  # 1024 columns per core
NLEV = 7
KS = [127 - (1 << k) for k in range(NLEV)]  # valid rows of level k

BF16 = mybir.dt.bfloat16
F32 = mybir.dt.float32

N_WARMUP = 8  # zero matmuls bridging kernel start -> first weights

_CACHE = {}

TRACE = False
LAST_RESULTS = None

# chunk c holds, per level k in CHUNK_LEVELS[c]: [sh_k | g(0,0,k) g(1,0,k)
# g(0,1,k) g(1,1,k)]  (sh only for k < 6)
CHUNK_LEVELS = [[0], [1, 2], [3, 4, 5, 6]]


def _wts_layout():
    """ft chunk + three weight chunks (one DMA each, ordered by level)."""
    offs = {}
    totals = []
    for levels in CHUNK_LEVELS:
        off = 0
        ci = len(totals) + 1
        for k in levels:
            if k < NLEV - 1:
                offs[("sh", k)] = (ci, off, KS[k + 1])
                off += KS[k + 1]
            for gi in range(2):
                for h in range(2):
                    if k == 0 and gi == 1:
                        # level-0 windows have len 1, so both lookups hit
                        # the same column: gi=1 reuses gi=0's one-hot
                        continue
                    offs[("g", gi, h, k)] = (ci, off, T)
                    off += T
        totals.append(off)
    return offs, totals


def _build_module():
    nc = bacc.Bacc(None, target_bir_lowering=False, debug=False)

    offs, totals = _wts_layout()
    # level-0 weights ride in the same DMA as the feature (bigger rows
    # stream faster, and level 0 gates the first real matmul)
    ft = nc.dram_tensor("ft", [T, CPRIME + totals[0]], BF16, kind="ExternalInput")
    wts = [
        nc.dram_tensor(f"w{c}", [T, totals[c]], BF16, kind="ExternalInput")
        for c in range(1, 3)
    ]
    out = nc.dram_tensor("out", [T, CPRIME], BF16, kind="ExternalOutput")

    with tile.TileContext(nc) as tc:
        with (
            tc.tile_pool(name="lv", bufs=1) as lvp,
            tc.tile_pool(name="gw", bufs=1) as gwp,
            tc.tile_pool(name="acc", bufs=1, space=bass.MemorySpace.PSUM) as accp,
            tc.tile_pool(name="shp", bufs=4, space=bass.MemorySpace.PSUM) as shpp,
        ):
            ftw = gwp.tile([T, CPRIME + totals[0]], BF16, name="ftw")
            wt12 = [
                gwp.tile([T, totals[c]], BF16, name=f"wt{c}") for c in (1, 2)
            ]
            wt = [ftw, wt12[0], wt12[1]]  # chunk index -> tile
            woff = [CPRIME, 0, 0]  # chunk base column within its tile
            # ft + level-0 weights in one SP-queue DMA (bigger rows stream
            # faster and level 0 gates the first real matmul); w1 on the
            # ACT queue, w2 (consumed last) on the Pool queue.
            nc.sync.dma_start(out=ftw[:, :], in_=ft[:, :])
            nc.scalar.dma_start(out=wt12[0][:, :], in_=wts[0][:, :])
            nc.gpsimd.dma_start(out=wt12[1][:, :], in_=wts[1][:, :])

            L = [ftw[:, 0:CPRIME]] + [
                lvp.tile([KS[k], CPRIME], BF16, name=f"L{k}")[:, :]
                for k in range(1, NLEV)
            ]

            def sh_ap(k):
                ci, o, n = offs[("sh", k)]
                o += woff[ci - 1]
                return wt[ci - 1][0 : KS[k], o : o + n]

            def g_ap(gi, h, k):
                if k == 0:
                    gi = 0  # level-0 one-hots are identical for both lookups
                ci, o, n = offs[("g", gi, h, k)]
                o += woff[ci - 1]
                return wt[ci - 1][0 : KS[k], o : o + n]

            p_acc = [
                accp.tile([T, CPRIME], F32, name=f"pacc{gi}") for gi in range(2)
            ]

            # PE warmup: HAM runs the PE at reduced clock until ~3us of
            # continuous busy; zero bf16 matmuls bridge the gap between
            # kernel start and the first weight DMA landing so the real
            # matmuls run at full clock with no idle gap.
            wzero = gwp.tile([128, 512], BF16, name="wzero")
            nc.vector.memset(wzero[:, :], 0.0)
            for _ in range(N_WARMUP):
                nc.tensor.matmul(
                    p_acc[0][:, 0:512],
                    wzero[0:126, 0:126],
                    wzero[0:126, 0:512],
                    start=True,
                    stop=True,
                )

            # per level: shift both halves first (PE), gathers accumulate,
            # DVE folds the shifted rows into the next level meanwhile.
            for k in range(NLEV):
                pe_shift = k < NLEV - 1
                for h in range(2) if k < NLEV - 1 else ():
                    sl = slice(h * 512, (h + 1) * 512)
                    if pe_shift:
                        shp = shpp.tile(
                            [KS[k + 1], 512], F32, name=f"shp{k}{h}", tag="shp"
                        )
                        nc.tensor.matmul(
                            shp[:, :],
                            sh_ap(k),
                            L[k][:, sl],
                            start=True,
                            stop=True,
                        )
                    if k < NLEV - 1:
                        for gi in range(2):
                            nc.tensor.matmul(
                                p_acc[gi][:, sl],
                                g_ap(gi, h, k),
                                L[k][:, sl],
                                start=(k == 0),
                                stop=False,
                            )
                    if pe_shift:
                        nc.vector.tensor_max(
                            L[k + 1][:, sl],
                            L[k][0 : KS[k + 1], sl],
                            shp[:, :],
                        )
                if k == NLEV - 1:
                    # last level, gi-major: acc0's final writes land first so
                    # the DVE staging op can start under the acc1 passes
                    for gi in range(2):
                        for h in range(2):
                            sl = slice(h * 512, (h + 1) * 512)
                            nc.tensor.matmul(
                                p_acc[gi][:, sl],
                                g_ap(gi, h, k),
                                L[k][:, sl],
                                start=False,
                                stop=True,
                            )

            # final per half (DVE can read only one PSUM operand per op):
            # stage acc0 to SBUF bf16 via tensor_scalar_max, then max with
            # acc1; one output DMA per half.
            s1t = gwp.tile([T, CPRIME], BF16, name="s1t")
            ot = gwp.tile([T, CPRIME], BF16, name="ot")
            for half in range(2):
                sl = slice(half * 512, (half + 1) * 512)
                nc.scalar.copy(s1t[:, sl], p_acc[0][:, sl])
            for half in range(2):
                sl = slice(half * 512, (half + 1) * 512)
                nc.vector.tensor_max(ot[:, sl], s1t[:, sl], p_acc[1][:, sl])
                e0 = nc.sync if half == 0 else nc.scalar
                e1 = nc.gpsimd if half == 0 else nc.sync
                e0.dma_start(out=out[0:63, sl], in_=ot[0:63, sl])
                e1.dma_start(out=out[63:126, sl], in_=ot[63:126, sl])

    nc.compile()
    return nc


def _host_windows(segments):
    """Replicates the reference's index math on segments[0]. Returns per half
    (lo, hi) clamped windows plus the empty mask."""
    seg = np.clip(segments.astype(np.float32), 0.0, 125.0)
    row = seg[0]  # [T, 4]
    s0 = np.floor(row[:, 0]).astype(np.int32)
    s1 = np.ceil(row[:, 1]).astype(np.int32)
    s1 = np.where(s0 == s1, s1 + 1, s1)
    e0 = np.floor(row[:, 2]).astype(np.int32)
    e1 = np.ceil(row[:, 3]).astype(np.int32)
    e0 = np.where(e0 == e1, e0 - 1, e0)

    halves = []
    for lo, hi in ((s0, s1), (e0, e1)):
        lo_c = np.maximum(lo, 0)
        hi_c = np.minimum(hi, T)
        empty = lo_c >= hi_c
        halves.append((lo_c, hi_c, empty))
    return halves


def _host_matrices(segments):
    halves = _host_windows(segments)
    g = {
        (gi, h, k): np.zeros((KS[k], T), np.float32)
        for gi in range(2)
        for h in range(2)
        for k in range(NLEV)
    }
    for h, (lo, hi, empty) in enumerate(halves):
        for t in range(T):
            if empty[t]:
                continue
            ln = int(hi[t] - lo[t])
            k = ln.bit_length() - 1
            g[(0, h, k)][int(lo[t]), t] = 1.0
            g[(1, h, k)][int(hi[t]) - (1 << k), t] = 1.0
    sh = {}
    for k in range(NLEV - 1):
        m = np.zeros((KS[k], KS[k + 1]), np.float32)
        s = 1 << k
        for j in range(KS[k + 1]):
            m[j + s, j] = 1.0
        sh[k] = m
    return g, sh, halves


def _shard_feature(feature):
    """Core i gets batches [2i, 2i+2) as [T, CPRIME] bf16 with
    c' = half*512 + local_batch*256 + channel_within_half."""
    fts = []
    for i in range(NCORES):
        pair = feature[BPC * i : BPC * (i + 1)]
        arr = pair.reshape(BPC, 2, C, T)  # [b, h, c, j]
        arr = np.ascontiguousarray(arr.transpose(3, 1, 0, 2).reshape(T, CPRIME))
        fts.append(arr.astype(ml_dtypes.bfloat16))
    return fts


def _unshard(results, halves):
    out = np.empty((B, C2, T), np.float32)
    for i in range(NCORES):
        r = np.asarray(results[i]["out"]).astype(np.float32)  # [T, CPRIME]
        arr = r.reshape(T, 2, BPC, C).transpose(2, 1, 3, 0)  # [b, h, c, t]
        out[BPC * i : BPC * (i + 1)] = arr.reshape(BPC, C2, T)
    neg = np.finfo(np.float32).min
    for h, (_, _, empty) in enumerate(halves):
        if empty.any():
            out[:, h * C : (h + 1) * C, empty] = neg
    return out


def kernel(feature, segments):
    global LAST_RESULTS
    feature = np.ascontiguousarray(feature, dtype=np.float32)
    segments = np.ascontiguousarray(segments, dtype=np.float32)

    g, sh, halves = _host_matrices(segments)
    if "nc" not in _CACHE:
        _CACHE["nc"] = _build_module()
    nc = _CACHE["nc"]

    fts = _shard_feature(feature)

    offs, totals = _wts_layout()
    chunks = [np.zeros((T, totals[c]), np.float32) for c in range(3)]
    for k in range(NLEV):
        if k < NLEV - 1:
            ci, o, n = offs[("sh", k)]
            chunks[ci - 1][: KS[k], o : o + n] = sh[k]
        for gi in range(2):
            for h in range(2):
                if k == 0 and gi == 1:
                    continue
                ci, o, n = offs[("g", gi, h, k)]
                chunks[ci - 1][: KS[k], o : o + n] = g[(gi, h, k)]
    chunks = [c.astype(ml_dtypes.bfloat16) for c in chunks]
    in_maps = []
    for i in range(NCORES):
        m = {
            "ft": np.ascontiguousarray(np.concatenate([fts[i], chunks[0]], axis=1)),
            "w1": chunks[1],
            "w2": chunks[2],
        }
        in_maps.append(m)

    res = run_bass_kernel_spmd(nc, in_maps, list(range(NCORES)), trace=TRACE)
    LAST_RESULTS = res
    return _unshard(res.results, halves)
